# revision 1
# baseline (speedup 1.0000x reference)
"""GraphSAGE (2-layer, MaxPool aggregator) on 8 Trainium2 NeuronCores.

Algorithm (per layer, exact rewrite of the reference):
    pooled = max_k relu(h[nbr] @ Wp + bp)  ==  relu(max_k T[nbr[:,k]] + bp),
    with T = h @ Wp computed ONCE per node (16x fewer FLOPs than reference).
    out = h @ W_top + pooled @ W_bot + b   (concat split into two matmuls)

Distribution: nodes sharded 8 ways (6272 padded rows/core). Every core
computes the full T1 table (inputs are replicated). pooled1^T (bf16) is
AllGathered so every core can build the full T2 table for the layer-2
neighbor gather; everything else is shard-local.

Layout: activations are kept feature-major ([feat, node]) for matmuls
(weights stationary); gather tables are node-major in DRAM. PE-transposes
bridge the two. Neighbor rows are fetched with per-(tile,k) indirect DMAs
(128 rows per call, one row per SBUF partition).
"""
import numpy as np
import ml_dtypes

import concourse.bass as bass
import concourse.bacc as bacc
import concourse.mybir as mybir
import concourse.tile as tile
from concourse.bass_utils import run_bass_kernel_spmd

CORES = 8
N, K, F0, F1, F2 = 50000, 16, 128, 256, 128
SH = 6272                    # padded shard rows per core (49 tiles of 128)
NP = SH * CORES              # 50176 padded total
TILES = SH // 128            # 49
FULL_TILES = NP // 128       # 392

_BUILD_CACHE = {}


def _build():
    if "nc" in _BUILD_CACHE:
        return _BUILD_CACHE["nc"]
    dt = mybir.dt
    nc = bacc.Bacc("TRN2", target_bir_lowering=False, debug=False,
                   enable_asserts=False, num_devices=CORES)
    # ---- I/O ----
    xT = nc.dram_tensor("xT", [128, NP], dt.bfloat16, kind="ExternalInput").ap()
    xTs = nc.dram_tensor("xTs", [128, SH], dt.bfloat16, kind="ExternalInput").ap()
    idx = nc.dram_tensor("idx", [128, TILES * K], dt.int32, kind="ExternalInput").ap()
    wp1 = nc.dram_tensor("wp1", [F0, F0], dt.bfloat16, kind="ExternalInput").ap()
    w1 = nc.dram_tensor("w1", [2 * F0, F1], dt.bfloat16, kind="ExternalInput").ap()
    wp2 = nc.dram_tensor("wp2", [F1, F1], dt.bfloat16, kind="ExternalInput").ap()
    w2 = nc.dram_tensor("w2", [2 * F1, F2], dt.bfloat16, kind="ExternalInput").ap()
    bp1 = nc.dram_tensor("bp1", [F0, 1], dt.float32, kind="ExternalInput").ap()
    b1 = nc.dram_tensor("b1", [F1, 1], dt.float32, kind="ExternalInput").ap()
    bp2 = nc.dram_tensor("bp2", [F1, 1], dt.float32, kind="ExternalInput").ap()
    b2b = nc.dram_tensor("b2b", [128, F2], dt.float32, kind="ExternalInput").ap()
    idf = nc.dram_tensor("idf", [128, 128], dt.float32, kind="ExternalInput").ap()
    idb = nc.dram_tensor("idb", [128, 128], dt.bfloat16, kind="ExternalInput").ap()
    out = nc.dram_tensor("out", [SH, F2], dt.float32, kind="ExternalOutput").ap()

    with tile.TileContext(nc) as tc:
        with (
            tc.tile_pool(name="cst", bufs=1) as cst,
            tc.tile_pool(name="wk", bufs=2) as wk,
            tc.tile_pool(name="ps", bufs=3, space="PSUM") as ps,
            tc.tile_pool(name="dram", bufs=1, space="DRAM") as dram,
        ):
            # ---- resident constants ----
            xT_sb = cst.tile([128, NP], dt.bfloat16)
            nc.sync.dma_start(xT_sb[:], xT)
            idx_sb = cst.tile([128, TILES * K], dt.int32)
            nc.sync.dma_start(idx_sb[:], idx)
            wp1_sb = cst.tile([128, F0], dt.bfloat16)
            nc.sync.dma_start(wp1_sb[:], wp1)
            w1_sb = cst.tile([128, 4 * 128], dt.bfloat16)  # [i*2+o] blocks
            for i in range(2):
                for o in range(2):
                    nc.sync.dma_start(w1_sb[:, (i * 2 + o) * 128:(i * 2 + o + 1) * 128],
                                      w1[i * 128:(i + 1) * 128, o * 128:(o + 1) * 128])
            wp2_sb = cst.tile([128, 2 * F1], dt.bfloat16)  # two [128,256] blocks
            for i in range(2):
                nc.sync.dma_start(wp2_sb[:, i * F1:(i + 1) * F1],
                                  wp2[i * 128:(i + 1) * 128, :])
            w2_sb = cst.tile([128, 4 * F2], dt.bfloat16)   # four [128,128] blocks
            for j in range(4):
                nc.sync.dma_start(w2_sb[:, j * F2:(j + 1) * F2],
                                  w2[j * 128:(j + 1) * 128, :])
            bp1_sb = cst.tile([128, 1], dt.float32)
            nc.sync.dma_start(bp1_sb[:], bp1)
            b1_sb = cst.tile([128, 2], dt.float32)
            nc.sync.dma_start(b1_sb[:, 0:1], b1[0:128, :])
            nc.sync.dma_start(b1_sb[:, 1:2], b1[128:256, :])
            bp2_sb = cst.tile([128, 2], dt.float32)
            nc.sync.dma_start(bp2_sb[:, 0:1], bp2[0:128, :])
            nc.sync.dma_start(bp2_sb[:, 1:2], bp2[128:256, :])
            b2b_sb = cst.tile([128, F2], dt.float32)
            nc.sync.dma_start(b2b_sb[:], b2b)
            idf_sb = cst.tile([128, 128], dt.float32)
            nc.sync.dma_start(idf_sb[:], idf)
            idb_sb = cst.tile([128, 128], dt.bfloat16)
            nc.sync.dma_start(idb_sb[:], idb)
            h1T_sh = cst.tile([128, 2 * SH], dt.bfloat16)  # my shard h1^T, 2 f-blocks

            # ---- DRAM scratch ----
            t1_dram = dram.tile([NP, F0], dt.float32)
            t2_dram = dram.tile([NP, F1], dt.bfloat16)
            p1T_dram = dram.tile([128, SH], dt.bfloat16)
            p1T_full = dram.tile([CORES, 128, SH], dt.bfloat16, addr_space="Shared")

            STG = 8  # tiles per staged table write

            # ============ Phase 1: T1 = x @ Wp1 (full, node-major fp32) ====
            for t0 in range(0, FULL_TILES, STG):
                nst = min(STG, FULL_TILES - t0)
                t1_stage = wk.tile([128, STG, F0], dt.float32)
                for j in range(nst):
                    t = t0 + j
                    ps_mm = ps.tile([128, 512], dt.float32, tag="mm")
                    nc.tensor.matmul(ps_mm[:, :F0],
                                     lhsT=xT_sb[:, t * 128:(t + 1) * 128],
                                     rhs=wp1_sb[:], start=True, stop=True)
                    nc.vector.tensor_copy(t1_stage[:, j, :], ps_mm[:, :F0])
                nc.sync.dma_start(
                    t1_dram[t0 * 128:(t0 + nst) * 128, :].rearrange(
                        "(t p) f -> p t f", p=128),
                    t1_stage[:, :nst, :])

            # ===== Phase 2: gather T1 rows, pooled1^T = relu(max + bp1) ====
            PSTG = 8
            for t0 in range(0, TILES, PSTG):
                nst = min(PSTG, TILES - t0)
                p1_stage = wk.tile([128, PSTG * 128], dt.bfloat16)
                for j in range(nst):
                    t = t0 + j
                    g1 = wk.tile([128, K * F0], dt.float32)
                    for k in range(K):
                        nc.gpsimd.indirect_dma_start(
                            out=g1[:, k * F0:(k + 1) * F0], out_offset=None,
                            in_=t1_dram[:],
                            in_offset=bass.IndirectOffsetOnAxis(
                                ap=idx_sb[:, t * K + k:t * K + k + 1], axis=0))
                    w = K * F0 // 2
                    while w >= F0:
                        nc.vector.tensor_max(out=g1[:, :w], in0=g1[:, :w],
                                             in1=g1[:, w:2 * w])
                        w //= 2
                    ps_tr = ps.tile([128, 128], dt.float32, tag="tr")
                    nc.tensor.transpose(ps_tr[:], g1[:, :F0], idf_sb[:])
                    nc.scalar.activation(p1_stage[:, j * 128:(j + 1) * 128], ps_tr[:],
                                         mybir.ActivationFunctionType.Relu,
                                         bias=bp1_sb[:], scale=1.0)
                nc.sync.dma_start(p1T_dram[:, t0 * 128:(t0 + nst) * 128],
                                  p1_stage[:, :nst * 128])

            # ============ Phase 3: AllGather pooled1^T ============
            nc.gpsimd.collective_compute(
                "AllGather", mybir.AluOpType.bypass,
                replica_groups=[list(range(CORES))],
                ins=[p1T_dram.opt()], outs=[p1T_full.opt()])

            # ==== Phase 4: h1^T (all nodes) and T2 = h1 @ Wp2 (node-major) ====
            CH = 512
            for r in range(CORES):
                for c0 in range(0, SH, CH):
                    n = min(CH, SH - c0)
                    p1c = wk.tile([128, CH], dt.bfloat16)
                    nc.sync.dma_start(p1c[:, :n], p1T_full[r, :, c0:c0 + n])
                    h1c = wk.tile([128, 2 * CH], dt.bfloat16)
                    for o in range(2):
                        ps_h = ps.tile([128, 512], dt.float32, tag="mm")
                        nc.tensor.matmul(ps_h[:, :n],
                                         lhsT=w1_sb[:, (0 * 2 + o) * 128:(0 * 2 + o + 1) * 128],
                                         rhs=xT_sb[:, r * SH + c0:r * SH + c0 + n],
                                         start=True, stop=False)
                        nc.tensor.matmul(ps_h[:, :n],
                                         lhsT=w1_sb[:, (1 * 2 + o) * 128:(1 * 2 + o + 1) * 128],
                                         rhs=p1c[:, :n], start=False, stop=True)
                        nc.scalar.activation(h1c[:, o * CH:o * CH + n], ps_h[:, :n],
                                             mybir.ActivationFunctionType.Relu,
                                             bias=b1_sb[:, o:o + 1], scale=1.0)
                    t2_stage = wk.tile([128, 4, F1], dt.bfloat16)
                    for j in range(n // 128):
                        ps_t2 = ps.tile([128, 512], dt.float32, tag="mm")
                        nc.tensor.matmul(ps_t2[:, :F1],
                                         lhsT=h1c[:, j * 128:(j + 1) * 128],
                                         rhs=wp2_sb[:, :F1], start=True, stop=False)
                        nc.tensor.matmul(ps_t2[:, :F1],
                                         lhsT=h1c[:, CH + j * 128:CH + (j + 1) * 128],
                                         rhs=wp2_sb[:, F1:], start=False, stop=True)
                        nc.vector.tensor_copy(t2_stage[:, j, :], ps_t2[:, :F1])
                    nc.sync.dma_start(
                        t2_dram[r * SH + c0:r * SH + c0 + n, :].rearrange(
                            "(t p) f -> p t f", p=128),
                        t2_stage[:, :n // 128, :])

            # ==== Phase 4b: my shard h1^T from local inputs (rank-agnostic) ====
            for c0 in range(0, SH, CH):
                n = min(CH, SH - c0)
                xsc = wk.tile([128, CH], dt.bfloat16)
                nc.sync.dma_start(xsc[:, :n], xTs[:, c0:c0 + n])
                p1s = wk.tile([128, CH], dt.bfloat16)
                nc.sync.dma_start(p1s[:, :n], p1T_dram[:, c0:c0 + n])
                for o in range(2):
                    ps_h2 = ps.tile([128, 512], dt.float32, tag="mm")
                    nc.tensor.matmul(ps_h2[:, :n],
                                     lhsT=w1_sb[:, (0 * 2 + o) * 128:(0 * 2 + o + 1) * 128],
                                     rhs=xsc[:, :n], start=True, stop=False)
                    nc.tensor.matmul(ps_h2[:, :n],
                                     lhsT=w1_sb[:, (1 * 2 + o) * 128:(1 * 2 + o + 1) * 128],
                                     rhs=p1s[:, :n], start=False, stop=True)
                    nc.scalar.activation(h1T_sh[:, o * SH + c0:o * SH + c0 + n],
                                         ps_h2[:, :n],
                                         mybir.ActivationFunctionType.Relu,
                                         bias=b1_sb[:, o:o + 1], scale=1.0)

            # ==== Phase 5: gather T2 rows, pooled2, out2 = [h1,p2] @ W2 + b2 ====
            OSTG = 8
            for t0 in range(0, TILES, OSTG):
                nst = min(OSTG, TILES - t0)
                o_stage = wk.tile([128, OSTG, F2], dt.float32)
                for j in range(nst):
                    t = t0 + j
                    g2 = wk.tile([128, K * F1], dt.bfloat16)
                    for k in range(K):
                        nc.gpsimd.indirect_dma_start(
                            out=g2[:, k * F1:(k + 1) * F1], out_offset=None,
                            in_=t2_dram[:],
                            in_offset=bass.IndirectOffsetOnAxis(
                                ap=idx_sb[:, t * K + k:t * K + k + 1], axis=0))
                    w = K * F1 // 2
                    while w >= F1:
                        nc.vector.tensor_max(out=g2[:, :w], in0=g2[:, :w],
                                             in1=g2[:, w:2 * w])
                        w //= 2
                    p2T = wk.tile([128, 2 * 128], dt.bfloat16)
                    for o in range(2):
                        ps_t = ps.tile([128, 128], dt.bfloat16, tag="tr")
                        nc.tensor.transpose(ps_t[:], g2[:, o * 128:(o + 1) * 128],
                                            idb_sb[:])
                        nc.scalar.activation(p2T[:, o * 128:(o + 1) * 128], ps_t[:],
                                             mybir.ActivationFunctionType.Relu,
                                             bias=bp2_sb[:, o:o + 1], scale=1.0)
                    ps_o = ps.tile([128, 512], dt.float32, tag="mm")
                    lhs_list = [h1T_sh[:, t * 128:(t + 1) * 128],
                                h1T_sh[:, SH + t * 128:SH + (t + 1) * 128],
                                p2T[:, :128], p2T[:, 128:]]
                    for jj in range(4):
                        nc.tensor.matmul(ps_o[:, :F2], lhsT=lhs_list[jj],
                                         rhs=w2_sb[:, jj * F2:(jj + 1) * F2],
                                         start=(jj == 0), stop=(jj == 3))
                    nc.vector.tensor_add(out=o_stage[:, j, :], in0=ps_o[:, :F2],
                                         in1=b2b_sb[:])
                nc.sync.dma_start(
                    out[t0 * 128:(t0 + nst) * 128, :].rearrange(
                        "(t p) f -> p t f", p=128),
                    o_stage[:, :nst, :])

    nc.compile()
    _BUILD_CACHE["nc"] = nc
    return nc


def prepare_in_maps(features, neighbor_idx, Wp1, bp1, W1, b1, Wp2, bp2, W2, b2):
    bf16 = ml_dtypes.bfloat16
    f = np.asarray(features, np.float32)
    nb = np.asarray(neighbor_idx).astype(np.int32)
    xpad = np.zeros((NP, F0), np.float32)
    xpad[:N] = f
    nbpad = np.zeros((NP, K), np.int32)
    nbpad[:N] = nb
    xT_np = np.ascontiguousarray(xpad.T).astype(bf16)
    idf_np = np.eye(128, dtype=np.float32)
    common = dict(
        xT=xT_np,
        wp1=np.asarray(Wp1, np.float32).astype(bf16),
        w1=np.asarray(W1, np.float32).astype(bf16),
        wp2=np.asarray(Wp2, np.float32).astype(bf16),
        w2=np.asarray(W2, np.float32).astype(bf16),
        bp1=np.asarray(bp1, np.float32).reshape(F0, 1),
        b1=np.asarray(b1, np.float32).reshape(F1, 1),
        bp2=np.asarray(bp2, np.float32).reshape(F1, 1),
        b2b=np.tile(np.asarray(b2, np.float32).reshape(1, F2), (128, 1)),
        idf=idf_np,
        idb=idf_np.astype(bf16),
    )
    in_maps = []
    for c in range(CORES):
        sl = nbpad[c * SH:(c + 1) * SH]              # [SH, K]
        idx_c = np.ascontiguousarray(
            sl.reshape(TILES, 128, K).transpose(1, 0, 2).reshape(128, TILES * K))
        xTs_c = np.ascontiguousarray(xT_np[:, c * SH:(c + 1) * SH])
        in_maps.append(dict(common, idx=idx_c, xTs=xTs_c))
    return in_maps


def kernel(features, neighbor_idx, Wp1, bp1, W1, b1, Wp2, bp2, W2, b2):
    in_maps = prepare_in_maps(features, neighbor_idx, Wp1, bp1, W1, b1,
                              Wp2, bp2, W2, b2)
    nc = _build()
    res = run_bass_kernel_spmd(nc, in_maps, core_ids=list(range(CORES)))
    full = np.concatenate([res.results[c]["out"] for c in range(CORES)], axis=0)
    return np.ascontiguousarray(full[:N]).astype(np.float32)

